# revision 1
# baseline (speedup 1.0000x reference)
"""Trainium2 Bass kernel for a DeepSeek-style MoE block (expert-parallel over 8 cores).

Strategy (dense expert-parallel):
  - Each core owns one expert (8 experts / 8 cores). x (transposed) + router
    weights are replicated; c_fc/c_proj are sharded along the expert axis.
  - Every core computes the full router on-device: logits -> top-2 -> softmax
    -> capacity ranking (exclusive cumsum over the flattened (slot, token)
    order via a strictly-triangular matmul + log-step block scan). The result
    is a dense per-token weight vector for this core's expert (0 for tokens
    not routed here or dropped by capacity).
  - Expert compute runs DENSELY over all 4096 tokens (2x the routed FLOPs,
    but no gathers/scatters — indirect DMA ops cost ~30us each on this stack
    and a permutation-based dispatch needs >100 of them). The per-token
    weight is applied to the expert output, which is written densely to a
    [N, D] partial buffer.
  - A ReduceScatter across the 8 cores combines partials; each core
    LayerNorms its 1/8 token shard and returns it. The host concatenates.

Matmul orientation: activations stay feature-major so both weights are used
in their native layout:
  hT[f, t] = sum_d c_fc[d, f] * xT[d, t]       (lhsT = c_fc slab, rhs = xT)
  eo[t, d] = sum_f hT[f, t] * c_proj[f, d]     (lhsT = hT slice,  rhs = c_proj slab)
"""

import os
import sys
from contextlib import ExitStack

import numpy as np

for _p in ("/opt/trn_rl_repo", "/root/.axon_site/_ro/trn_rl_repo"):
    if os.path.isdir(_p) and _p not in sys.path:
        sys.path.insert(0, _p)

P = 128

FULL_CFG = dict(N=4096, D=1024, E=8, CAP=2048, TB=1024, n_cores=8,
                act="Gelu", ln_eps=1e-5)


def build_moe_kernel(N, D, E, CAP, TB, n_cores, act="Gelu", ln_eps=1e-5,
                     debug_taps=False, stages=99, dbg_sub="", repeat=1):
    """Builds and compiles the SPMD Bass kernel. Returns the Bacc object.

    stages (perf bisection): 0=router only, 1=+mm1, 2=+mm2/partial,
    3=+ReduceScatter, 4=full (LN).
    """
    from concourse import bacc, bass, mybir
    import concourse.tile as tile
    from concourse.masks import make_identity, make_upper_triangular

    FP32 = mybir.dt.float32
    BF16 = mybir.dt.bfloat16
    AF = mybir.ActivationFunctionType
    ALU = mybir.AluOpType
    X = mybir.AxisListType.X

    F = 4 * D
    NCH = N // P           # token chunks
    KD = D // P            # contraction chunks for mm1
    FCH = F // P           # f chunks
    B2 = 2 * NCH           # (slot k, token-chunk) columns in rank order
    NTB = N // TB          # token blocks for the expert pipeline
    MCH = TB // P          # token chunks per block
    DHW = min(512, D)      # mm2 output width per matmul
    NDH = D // DHW
    HHW = min(512, TB)     # mm1 output width per matmul
    NHH = TB // HHW
    NSH = N // n_cores     # output shard rows per core
    NB512 = N // 512       # router column blocks
    act_fn = getattr(AF, act)
    assert N % 512 == 0 and B2 * E <= 512
    subs = set(dbg_sub.split(",")) if dbg_sub else set()

    nc = bacc.Bacc("TRN2", target_bir_lowering=False, debug=False,
                   num_devices=n_cores)

    xT = nc.dram_tensor("xT", [D, N], FP32, kind="ExternalInput").ap()
    wg = nc.dram_tensor("wg", [D, E], FP32, kind="ExternalInput").ap()
    cfc = nc.dram_tensor("cfc", [FCH, P, KD, P], FP32, kind="ExternalInput").ap()
    cpj = nc.dram_tensor("cpj", [NDH, FCH, P, DHW], BF16, kind="ExternalInput").ap()
    esel = nc.dram_tensor("esel", [P, B2 * E], FP32, kind="ExternalInput").ap()
    lnw = nc.dram_tensor("lnw", [P, D], FP32, kind="ExternalInput").ap()
    lnb = nc.dram_tensor("lnb", [P, D], FP32, kind="ExternalInput").ap()
    out_ext = nc.dram_tensor("out", [NSH, D], FP32, kind="ExternalOutput").ap()
    if debug_taps:
        dbg_logits = nc.dram_tensor("dbg_logits", [P, NCH * E], FP32,
                                    kind="ExternalOutput").ap()
        dbg_rnk = nc.dram_tensor("dbg_rnk", [P, B2 * E], FP32,
                                 kind="ExternalOutput").ap()
        dbg_wden = nc.dram_tensor("dbg_wden", [P, NCH], FP32,
                                  kind="ExternalOutput").ap()
        dbg_partial = nc.dram_tensor("dbg_partial", [N, D], FP32,
                                     kind="ExternalOutput").ap()
        dbg_rs = nc.dram_tensor("dbg_rs", [NSH, D], FP32,
                                kind="ExternalOutput").ap()

    with tile.TileContext(nc) as tc:
      with ExitStack() as root:
        dram = root.enter_context(tc.tile_pool(name="dram", bufs=1, space="DRAM"))
        ps = root.enter_context(tc.tile_pool(name="ps", bufs=8, space="PSUM"))
        const = root.enter_context(tc.tile_pool(name="const", bufs=1))
        rt = root.enter_context(tc.tile_pool(name="rt", bufs=1))
        mn = root.enter_context(tc.tile_pool(name="mn", bufs=1))
        lnp = root.enter_context(tc.tile_pool(name="ln", bufs=1))
        for _rep in range(repeat):

            partial = dram.tile([N, D], FP32)
            rs_out = dram.tile([NSH, D], FP32)

            ident = const.tile([P, P], FP32)
            make_identity(nc, ident[:])
            ustrict = const.tile([P, P], FP32)   # U[k, m] = 1 iff m > k
            make_upper_triangular(nc, ustrict[:], val=1.0, diag=False)
            ones_t = const.tile([P, P], FP32)
            nc.vector.memset(ones_t[:], 1.0)

            wden = const.tile([P, NCH], FP32)   # per-token weight, this expert

            # ---------------- router ----------------
            wg_sb = rt.tile([P, KD, E], FP32)
            nc.sync.dma_start(out=wg_sb[:], in_=wg.rearrange("(k p) e -> p k e", p=P))
            es_sb = rt.tile([P, B2 * E], FP32)
            nc.sync.dma_start(out=es_sb[:], in_=esel[:])

            # logits[n, e] computed as (w_g^T @ x^T)^T in 512-token blocks
            logits = rt.tile([P, NCH, E], FP32)
            for nb in range(NB512):
                ps_lt = ps.tile([P, 512], FP32, tag="ps")
                for k in range(KD):
                    xt_sb = rt.tile([P, 512], FP32, tag="xt", bufs=2)
                    nc.sync.dma_start(out=xt_sb[:], in_=xT[k * P:(k + 1) * P,
                                                         nb * 512:(nb + 1) * 512])
                    nc.tensor.matmul(out=ps_lt[:E, :], lhsT=wg_sb[:, k, :],
                                     rhs=xt_sb[:], start=(k == 0), stop=(k == KD - 1))
                lt_sb = rt.tile([E, 512], FP32, tag="lt", bufs=2)
                nc.vector.tensor_copy(out=lt_sb[:], in_=ps_lt[:E, :])
                for i in range(4):  # 512 tokens -> 4 chunks of 128
                    ps_t = ps.tile([P, 512], FP32, tag="ps")
                    nc.tensor.transpose(out=ps_t[:, :E], in_=lt_sb[:, i * P:(i + 1) * P],
                                        identity=ident[:E, :E])
                    nc.vector.tensor_copy(out=logits[:, nb * 4 + i, :], in_=ps_t[:, :E])

            if debug_taps:
                nc.sync.dma_start(out=dbg_logits[:],
                                  in_=logits[:].rearrange("p a e -> p (a e)"))
            # top-2 over experts
            v0 = rt.tile([P, NCH], FP32)
            nc.vector.tensor_reduce(out=v0[:], in_=logits[:], axis=X, op=ALU.max)
            mask01 = rt.tile([P, B2, E], FP32)
            nc.vector.tensor_tensor(out=mask01[:, :NCH, :], in0=logits[:],
                                    in1=v0[:].unsqueeze(2).to_broadcast([P, NCH, E]),
                                    op=ALU.is_equal)
            mbig = rt.tile([P, NCH, E], FP32)
            nc.vector.tensor_scalar(out=mbig[:], in0=mask01[:, :NCH, :],
                                    scalar1=1e30, scalar2=None, op0=ALU.mult)
            lm = rt.tile([P, NCH, E], FP32)
            nc.vector.tensor_tensor(out=lm[:], in0=logits[:], in1=mbig[:], op=ALU.subtract)
            v1 = rt.tile([P, NCH], FP32)
            nc.vector.tensor_reduce(out=v1[:], in_=lm[:], axis=X, op=ALU.max)
            nc.vector.tensor_tensor(out=mask01[:, NCH:, :], in0=lm[:],
                                    in1=v1[:].unsqueeze(2).to_broadcast([P, NCH, E]),
                                    op=ALU.is_equal)

            # softmax over the two selected logits
            dv = rt.tile([P, NCH], FP32)
            nc.vector.tensor_tensor(out=dv[:], in0=v1[:], in1=v0[:], op=ALU.subtract)
            p1 = rt.tile([P, NCH], FP32)
            nc.scalar.activation(out=p1[:], in_=dv[:], func=AF.Exp)
            z = rt.tile([P, NCH], FP32)
            nc.vector.tensor_scalar(out=z[:], in0=p1[:], scalar1=1.0, scalar2=None,
                                    op0=ALU.add)
            vw = rt.tile([P, B2], FP32)
            w0v = rt.tile([P, NCH], FP32)
            nc.vector.reciprocal(out=w0v[:], in_=z[:])
            nc.vector.tensor_copy(out=vw[:, :NCH], in_=w0v[:])
            nc.vector.tensor_tensor(out=vw[:, NCH:], in0=p1[:], in1=w0v[:], op=ALU.mult)

            # exclusive cumsum over flattened (k, n) per expert:
            # intra-chunk via strictly-upper-triangular matmul, chunk offsets
            # via a log-step scan over per-chunk column sums
            ps_s = ps.tile([P, 512], FP32, tag="ps")
            nc.tensor.matmul(out=ps_s[:, :B2 * E], lhsT=ustrict[:], rhs=mask01[:],
                             start=True, stop=True)
            ps_c = ps.tile([P, 512], FP32, tag="ps")
            nc.tensor.matmul(out=ps_c[:, :B2 * E], lhsT=ones_t[:], rhs=mask01[:],
                             start=True, stop=True)
            ea = rt.tile([P, B2 * E], FP32)
            eb2 = rt.tile([P, B2 * E], FP32)
            nc.vector.memset(ea[:, :E], 0.0)
            nc.vector.tensor_copy(out=ea[:, E:], in_=ps_c[:, :(B2 - 1) * E])
            cur, nxt = ea, eb2
            s = 1
            while s < B2:
                w = s * E
                nc.vector.tensor_copy(out=nxt[:, :w], in_=cur[:, :w])
                nc.vector.tensor_tensor(out=nxt[:, w:B2 * E], in0=cur[:, w:B2 * E],
                                        in1=cur[:, :B2 * E - w], op=ALU.add)
                cur, nxt = nxt, cur
                s *= 2
            rnk = rt.tile([P, B2 * E], FP32)
            nc.vector.tensor_tensor(out=rnk[:], in0=ps_s[:, :B2 * E], in1=cur[:],
                                    op=ALU.add)
            if debug_taps:
                nc.sync.dma_start(out=dbg_rnk[:], in_=rnk[:])

            # dense per-token weight for this core's expert:
            #   wden[n] = sum_k vw[k, n] * mask01[k, n, e0] * (rank < CAP)
            klt = rt.tile([P, B2 * E], FP32)
            nc.vector.tensor_scalar(out=klt[:], in0=rnk[:], scalar1=float(CAP),
                                    scalar2=None, op0=ALU.is_lt)
            kept = rt.tile([P, B2 * E], FP32)
            nc.vector.tensor_tensor(out=kept[:], in0=klt[:],
                                    in1=mask01[:].rearrange("p b e -> p (b e)"),
                                    op=ALU.mult)
            ksel = rt.tile([P, B2 * E], FP32)
            nc.vector.tensor_tensor(out=ksel[:], in0=kept[:], in1=es_sb[:], op=ALU.mult)
            ks2 = rt.tile([P, B2], FP32)
            nc.vector.tensor_reduce(out=ks2[:], in_=ksel[:].rearrange("p (b e) -> p b e", e=E),
                                    axis=X, op=ALU.add)
            wdb = rt.tile([P, B2], FP32)
            nc.vector.tensor_tensor(out=wdb[:], in0=ks2[:], in1=vw[:], op=ALU.mult)
            nc.vector.tensor_tensor(out=wden[:], in0=wdb[:, :NCH], in1=wdb[:, NCH:],
                                    op=ALU.add)
            if debug_taps:
                nc.sync.dma_start(out=dbg_wden[:], in_=wden[:])

            # ---------------- dense expert compute ----------------
            if True:
                hT = mn.tile([P, FCH, TB], BF16)
                for tb in range(NTB if stages >= 1 else 0):
                    xt_blk = mn.tile([P, KD, TB], FP32, tag="xtb", bufs=1)
                    for kd in range(KD):
                        nc.sync.dma_start(out=xt_blk[:, kd, :],
                                          in_=xT[kd * P:(kd + 1) * P,
                                                tb * TB:(tb + 1) * TB])
                    # mm1: hT = act(c_fc^T-contraction with xT), f-major
                    for f in range(FCH):
                        cfc_sb = mn.tile([P, KD, P], FP32, tag="cfc", bufs=3)
                        nc.sync.dma_start(out=cfc_sb[:], in_=cfc[f])
                        hps = [ps.tile([P, 512], FP32, tag="ps", name=f"hps{hh}")
                               for hh in range(NHH)]
                        for kd in range(KD):
                            for hh in range(NHH):
                                nc.tensor.matmul(out=hps[hh][:, :HHW], lhsT=cfc_sb[:, kd, :],
                                                 rhs=xt_blk[:, kd, hh * HHW:(hh + 1) * HHW],
                                                 start=(kd == 0), stop=(kd == KD - 1))
                        for hh in range(NHH):
                            nc.scalar.activation(out=hT[:, f, hh * HHW:(hh + 1) * HHW],
                                                 in_=hps[hh][:, :HHW], func=act_fn)
                    # mm2: eo accumulated over f, weighted, written densely
                    for dh in range(NDH if stages >= 2 else 0):
                        eops = [ps.tile([P, 512], FP32, tag="ps", name=f"eops{m}")
                                for m in range(MCH)]
                        for f in range(FCH):
                            cp = mn.tile([P, DHW], BF16, tag="cpj", bufs=3)
                            nc.sync.dma_start(out=cp[:], in_=cpj[dh, f])
                            for m in range(MCH):
                                nc.tensor.matmul(out=eops[m][:, :DHW],
                                                 lhsT=hT[:, f, m * P:(m + 1) * P],
                                                 rhs=cp[:],
                                                 start=(f == 0), stop=(f == FCH - 1))
                        for m in range(MCH):
                            g = tb * MCH + m
                            eo = mn.tile([P, DHW], FP32, tag="eo", bufs=2)
                            nc.vector.tensor_tensor(
                                out=eo[:], in0=eops[m][:, :DHW],
                                in1=wden[:, g:g + 1].to_broadcast([P, DHW]),
                                op=ALU.mult)
                            nc.sync.dma_start(
                                out=partial[g * P:(g + 1) * P, dh * DHW:(dh + 1) * DHW],
                                in_=eo[:])

            # ---------------- combine + layernorm ----------------
            if debug_taps:
                nc.sync.dma_start(out=dbg_partial[:], in_=partial[:])
            if stages >= 3:
                nc.gpsimd.collective_compute(
                    "ReduceScatter", mybir.AluOpType.add,
                    replica_groups=[list(range(n_cores))],
                    ins=[partial.opt()], outs=[rs_out.opt()])
            if debug_taps:
                nc.sync.dma_start(out=dbg_rs[:], in_=rs_out[:])
            if stages < 4:
                zo = const.tile([P, D], FP32)
                nc.vector.memset(zo[:], 0.0)
                for i in range((NSH + P - 1) // P):
                    rows = min(P, NSH - i * P)
                    nc.sync.dma_start(out=out_ext[i * P:i * P + rows, :],
                                      in_=zo[:rows, :])
            if stages >= 4:
                lnw_sb = lnp.tile([P, D], FP32)
                nc.sync.dma_start(out=lnw_sb[:], in_=lnw[:])
                lnb_sb = lnp.tile([P, D], FP32)
                nc.sync.dma_start(out=lnb_sb[:], in_=lnb[:])
                epsb = lnp.tile([P, 1], FP32)
                nc.vector.memset(epsb[:], float(ln_eps))
                nt = (NSH + P - 1) // P
                for i in range(nt):
                    rows = min(P, NSH - i * P)
                    xr = lnp.tile([P, D], FP32, tag="xr", bufs=1)
                    nc.sync.dma_start(out=xr[:rows, :], in_=rs_out[i * P:i * P + rows, :])
                    sm = lnp.tile([P, 1], FP32, tag="sm", bufs=1)
                    nc.vector.tensor_reduce(out=sm[:rows], in_=xr[:rows, :], axis=X, op=ALU.add)
                    mu = lnp.tile([P, 1], FP32, tag="mu", bufs=1)
                    nc.vector.tensor_scalar(out=mu[:rows], in0=sm[:rows], scalar1=1.0 / D,
                                            scalar2=None, op0=ALU.mult)
                    xc = lnp.tile([P, D], FP32, tag="xc", bufs=1)
                    nc.vector.tensor_scalar(out=xc[:rows], in0=xr[:rows, :], scalar1=mu[:rows],
                                            scalar2=None, op0=ALU.subtract)
                    vs = lnp.tile([P, 1], FP32, tag="vs", bufs=1)
                    nc.scalar.activation(out=xr[:rows, :], in_=xc[:rows], func=AF.Square,
                                         accum_out=vs[:rows])
                    vr = lnp.tile([P, 1], FP32, tag="vr", bufs=1)
                    nc.vector.tensor_scalar(out=vr[:rows], in0=vs[:rows], scalar1=1.0 / D,
                                            scalar2=None, op0=ALU.mult)
                    sd = lnp.tile([P, 1], FP32, tag="sd", bufs=1)
                    nc.scalar.activation(out=sd[:rows], in_=vr[:rows], func=AF.Sqrt,
                                         bias=epsb[:rows])
                    rsd = lnp.tile([P, 1], FP32, tag="rsd", bufs=1)
                    nc.vector.reciprocal(out=rsd[:rows], in_=sd[:rows])
                    yo = lnp.tile([P, D], FP32, tag="yo", bufs=1)
                    nc.vector.tensor_scalar(out=yo[:rows], in0=xc[:rows], scalar1=rsd[:rows],
                                            scalar2=None, op0=ALU.mult)
                    nc.vector.tensor_tensor(out=yo[:rows], in0=yo[:rows], in1=lnw_sb[:rows, :],
                                            op=ALU.mult)
                    nc.vector.tensor_tensor(out=yo[:rows], in0=yo[:rows], in1=lnb_sb[:rows, :],
                                            op=ALU.add)
                    nc.sync.dma_start(out=out_ext[i * P:i * P + rows, :], in_=yo[:rows, :])

    nc.compile()
    return nc


def prep_in_maps(x, w_g, c_fc, c_proj, ln_w, ln_b, cfg):
    """Host-side input prep: replication, layout tiling, bf16 cast."""
    from concourse import mybir

    N, D, E, CAP = cfg["N"], cfg["D"], cfg["E"], cfg["CAP"]
    n_cores = cfg["n_cores"]
    F = 4 * D
    KD, FCH = D // P, F // P
    NCH = N // P
    B2 = 2 * NCH
    DHW = min(512, D)
    NDH = D // DHW
    bf16 = mybir.dt.np(mybir.dt.bfloat16)

    xf = np.ascontiguousarray(np.asarray(x, np.float32).reshape(N, D))
    xT = np.ascontiguousarray(xf.T)
    wg = np.ascontiguousarray(np.asarray(w_g, np.float32))
    cfc_all = np.asarray(c_fc, np.float32)
    cpj_all = np.asarray(c_proj, np.float32)
    lnw = np.ascontiguousarray(np.broadcast_to(np.asarray(ln_w, np.float32), (P, D)))
    lnb = np.ascontiguousarray(np.broadcast_to(np.asarray(ln_b, np.float32), (P, D)))

    in_maps = []
    for e in range(n_cores):
        cfc_t = np.ascontiguousarray(
            cfc_all[e].reshape(KD, P, FCH, P).transpose(2, 1, 0, 3))
        cpj_t = np.ascontiguousarray(
            cpj_all[e].reshape(FCH, P, NDH, DHW).transpose(2, 0, 1, 3)).astype(bf16)
        ev = np.zeros((E,), np.float32)
        ev[e] = 1.0
        esel = np.ascontiguousarray(
            np.broadcast_to(np.tile(ev, B2), (P, B2 * E)))
        in_maps.append(dict(xT=xT, wg=wg, cfc=cfc_t, cpj=cpj_t,
                            esel=esel, lnw=lnw, lnb=lnb))
    return in_maps


_CACHE = {}


def _compiled_full():
    key = "full"
    if key not in _CACHE:
        _CACHE[key] = build_moe_kernel(**FULL_CFG)
    return _CACHE[key]


def run_on_hw(inputs, trace=False):
    """Runs the full-size kernel on the 8 NeuronCores. Returns (out, results)."""
    from concourse.bass_utils import run_bass_kernel_spmd

    cfg = FULL_CFG
    nc = _compiled_full()
    in_maps = prep_in_maps(inputs["x"], inputs["w_g"], inputs["c_fc"],
                           inputs["c_proj"], inputs["ln_w"], inputs["ln_b"], cfg)
    res = run_bass_kernel_spmd(nc, in_maps, list(range(cfg["n_cores"])),
                               trace=trace)
    shards = [res.results[i]["out"] for i in range(cfg["n_cores"])]
    out = np.concatenate(shards, axis=0).astype(np.float32)
    B, T = 4, 1024
    return out.reshape(B, T, cfg["D"]), res


def kernel(x, w_g, c_fc, c_proj, ln_w, ln_b):
    out, _ = run_on_hw(dict(x=x, w_g=w_g, c_fc=c_fc, c_proj=c_proj,
                            ln_w=ln_w, ln_b=ln_b))
    return out



# revision 2
# speedup vs baseline: 2.2871x; 2.2871x over previous
"""Trainium2 Bass kernel for a DeepSeek-style MoE block (expert-parallel over 8 cores).

Strategy (dense expert-parallel, bf16 expert compute):
  - Each core owns one expert (8 experts / 8 cores). x (transposed) + router
    weights are replicated; c_fc/c_proj are sharded along the expert axis.
  - Every core computes the full router on-device in fp32: logits -> top-2 ->
    softmax -> capacity ranking (exclusive cumsum over the flattened
    (slot, token) order via a strictly-triangular matmul + log-step block
    scan). The result is a dense per-token weight vector for this core's
    expert (0 for tokens not routed here or dropped by capacity).
  - Expert compute runs DENSELY over all 4096 tokens in bf16 (2x the routed
    FLOPs, but no gathers/scatters; bf16 runs the PE at 1 cycle/row vs 4 for
    fp32). Both weight matrices live SBUF-resident (8 MB + 8 MB bf16), loaded
    once. Tokens stream through in 8 blocks of 512.
  - The per-token weight is applied to the expert output, cast to bf16, and
    written densely to a per-block [512, D] partial buffer. A per-block
    ReduceScatter (bf16) combines partials across the 8 cores while later
    blocks still compute; each core LayerNorms its 64-row shard of every
    block. The host reassembles the full output.

Matmul orientation keeps activations feature-major so both weights are used
in their native layout:
  hT[f, t] = sum_d c_fc[d, f] * xT[d, t]       (lhsT = c_fc slab, rhs = xT)
  eo[t, d] = sum_f hT[f, t] * c_proj[f, d]     (lhsT = hT slice,  rhs = c_proj)
"""

import os
import sys
from contextlib import ExitStack

import numpy as np

for _p in ("/opt/trn_rl_repo", "/root/.axon_site/_ro/trn_rl_repo"):
    if os.path.isdir(_p) and _p not in sys.path:
        sys.path.insert(0, _p)

P = 128

FULL_CFG = dict(N=4096, D=1024, E=8, CAP=2048, TB=512, n_cores=8,
                act="Gelu", ln_eps=1e-5)


def build_moe_kernel(N, D, E, CAP, TB, n_cores, act="Gelu", ln_eps=1e-5,
                     debug_taps=False):
    """Builds and compiles the SPMD Bass kernel. Returns the Bacc object."""
    from concourse import bacc, bass, mybir
    import concourse.tile as tile
    from concourse.masks import make_identity, make_upper_triangular

    FP32 = mybir.dt.float32
    BF16 = mybir.dt.bfloat16
    AF = mybir.ActivationFunctionType
    ALU = mybir.AluOpType
    X = mybir.AxisListType.X

    F = 4 * D
    NCH = N // P           # token chunks (128 tokens each)
    KD = D // P            # contraction chunks for mm1
    FCH = F // P           # f chunks
    B2 = 2 * NCH           # (slot k, token-chunk) columns in rank order
    NTB = N // TB          # token blocks for the expert pipeline
    MCH = TB // P          # token chunks per block
    DHW = min(512, D)      # mm2 output width per matmul
    NDH = D // DHW
    HHW = min(512, TB)     # mm1 output width per matmul
    NHH = TB // HHW
    SH = TB // n_cores     # RS shard rows per core per block
    NB512 = N // 512       # router column blocks
    act_fn = getattr(AF, act)
    assert N % 512 == 0 and B2 * E <= 512 and NHH == 1

    nc = bacc.Bacc("TRN2", target_bir_lowering=False, debug=False,
                   num_devices=n_cores)

    xT = nc.dram_tensor("xT", [D, N], FP32, kind="ExternalInput").ap()
    wg = nc.dram_tensor("wg", [D, E], FP32, kind="ExternalInput").ap()
    xbh = nc.dram_tensor("xbh", [P, NTB, KD, TB], BF16, kind="ExternalInput").ap()
    cfc = nc.dram_tensor("cfc", [P, KD, FCH, P], BF16, kind="ExternalInput").ap()
    cpj = nc.dram_tensor("cpj", [P, FCH, D], BF16, kind="ExternalInput").ap()
    esel = nc.dram_tensor("esel", [P, B2 * E], FP32, kind="ExternalInput").ap()
    lnw = nc.dram_tensor("lnw", [P, D], BF16, kind="ExternalInput").ap()
    lnb = nc.dram_tensor("lnb", [P, D], BF16, kind="ExternalInput").ap()
    out_ext = nc.dram_tensor("out", [NTB * SH, D], FP32, kind="ExternalOutput").ap()
    if debug_taps:
        dbg_wden = nc.dram_tensor("dbg_wden", [P, NCH], FP32,
                                  kind="ExternalOutput").ap()
        dbg_partial = nc.dram_tensor("dbg_partial", [N, D], FP32,
                                     kind="ExternalOutput").ap()

    with tile.TileContext(nc) as tc:
      with ExitStack() as root:
        dram = root.enter_context(tc.tile_pool(name="dram", bufs=1, space="DRAM"))
        ps = root.enter_context(tc.tile_pool(name="ps", bufs=8, space="PSUM"))
        const = root.enter_context(tc.tile_pool(name="const", bufs=1))
        wts = root.enter_context(tc.tile_pool(name="wts", bufs=1))
        xbp = root.enter_context(tc.tile_pool(name="xbp", bufs=1))

        partial_b = [dram.tile([TB, D], BF16, name=f"partialb{tb}",
                               tag=f"pb{tb}") for tb in range(NTB)]
        rs_o = [dram.tile([SH, D], BF16, name=f"rso{tb}", tag=f"rs{tb}")
                for tb in range(NTB)]

        ident = const.tile([P, P], FP32)
        make_identity(nc, ident[:])
        ustrict = const.tile([P, P], FP32)   # U[k, m] = 1 iff m > k
        make_upper_triangular(nc, ustrict[:], val=1.0, diag=False)
        ones_t = const.tile([P, P], FP32)
        nc.vector.memset(ones_t[:], 1.0)
        wden = const.tile([P, NCH], FP32)    # per-token weight, this expert

        # -------- resident weights: load once, chunked across DMA rings ----
        cfc_sb = wts.tile([P, KD, FCH, P], BF16, tag="cfc")
        for kd in range(KD):
            nc.sync.dma_start(out=cfc_sb[:, kd], in_=cfc[:, kd])
        cpj_sb = wts.tile([P, FCH, D], BF16, tag="cpj")
        for fg in range(FCH // 2):
            nc.sync.dma_start(out=cpj_sb[:, fg * 2:(fg + 1) * 2],
                              in_=cpj[:, fg * 2:(fg + 1) * 2])
        # prefetch x block 0 for mm1
        xb_t = [None] * NTB
        xb_t[0] = xbp.tile([P, KD, TB], BF16, tag="xb", bufs=2, name="xb0")
        nc.sync.dma_start(out=xb_t[0][:], in_=xbh[:, 0])

        # ---------------- router (fp32, scoped pool) ----------------
        with tc.tile_pool(name="rt", bufs=1) as rt:
            wg_sb = rt.tile([P, KD, E], FP32)
            nc.sync.dma_start(out=wg_sb[:], in_=wg.rearrange("(k p) e -> p k e", p=P))
            es_sb = rt.tile([P, B2 * E], FP32)
            nc.sync.dma_start(out=es_sb[:], in_=esel[:])

            # logits[n, e] computed as (w_g^T @ x^T)^T in 512-token blocks
            logits = rt.tile([P, NCH, E], FP32)
            for nb in range(NB512):
                ps_lt = ps.tile([P, 512], FP32, tag="ps")
                for k in range(KD):
                    xt_sb = rt.tile([P, 512], FP32, tag="xt", bufs=2)
                    nc.sync.dma_start(out=xt_sb[:], in_=xT[k * P:(k + 1) * P,
                                                         nb * 512:(nb + 1) * 512])
                    nc.tensor.matmul(out=ps_lt[:E, :], lhsT=wg_sb[:, k, :],
                                     rhs=xt_sb[:], start=(k == 0), stop=(k == KD - 1))
                lt_sb = rt.tile([E, 512], FP32, tag="lt", bufs=2)
                nc.vector.tensor_copy(out=lt_sb[:], in_=ps_lt[:E, :])
                for i in range(4):  # 512 tokens -> 4 chunks of 128
                    ps_t = ps.tile([P, 512], FP32, tag="ps")
                    nc.tensor.transpose(out=ps_t[:, :E], in_=lt_sb[:, i * P:(i + 1) * P],
                                        identity=ident[:E, :E])
                    nc.vector.tensor_copy(out=logits[:, nb * 4 + i, :], in_=ps_t[:, :E])

            # top-2 over experts
            v0 = rt.tile([P, NCH], FP32)
            nc.vector.tensor_reduce(out=v0[:], in_=logits[:], axis=X, op=ALU.max)
            mask01 = rt.tile([P, B2, E], FP32)
            nc.vector.tensor_tensor(out=mask01[:, :NCH, :], in0=logits[:],
                                    in1=v0[:].unsqueeze(2).to_broadcast([P, NCH, E]),
                                    op=ALU.is_equal)
            mbig = rt.tile([P, NCH, E], FP32)
            nc.vector.tensor_scalar(out=mbig[:], in0=mask01[:, :NCH, :],
                                    scalar1=1e30, scalar2=None, op0=ALU.mult)
            lm = rt.tile([P, NCH, E], FP32)
            nc.vector.tensor_tensor(out=lm[:], in0=logits[:], in1=mbig[:], op=ALU.subtract)
            v1 = rt.tile([P, NCH], FP32)
            nc.vector.tensor_reduce(out=v1[:], in_=lm[:], axis=X, op=ALU.max)
            nc.vector.tensor_tensor(out=mask01[:, NCH:, :], in0=lm[:],
                                    in1=v1[:].unsqueeze(2).to_broadcast([P, NCH, E]),
                                    op=ALU.is_equal)

            # softmax over the two selected logits
            dv = rt.tile([P, NCH], FP32)
            nc.vector.tensor_tensor(out=dv[:], in0=v1[:], in1=v0[:], op=ALU.subtract)
            p1 = rt.tile([P, NCH], FP32)
            nc.scalar.activation(out=p1[:], in_=dv[:], func=AF.Exp)
            z = rt.tile([P, NCH], FP32)
            nc.vector.tensor_scalar(out=z[:], in0=p1[:], scalar1=1.0, scalar2=None,
                                    op0=ALU.add)
            vw = rt.tile([P, B2], FP32)
            w0v = rt.tile([P, NCH], FP32)
            nc.vector.reciprocal(out=w0v[:], in_=z[:])
            nc.vector.tensor_copy(out=vw[:, :NCH], in_=w0v[:])
            nc.vector.tensor_tensor(out=vw[:, NCH:], in0=p1[:], in1=w0v[:], op=ALU.mult)

            # exclusive cumsum over flattened (k, n) per expert:
            # intra-chunk via strictly-upper-triangular matmul, chunk offsets
            # via a log-step scan over per-chunk column sums
            ps_s = ps.tile([P, 512], FP32, tag="ps")
            nc.tensor.matmul(out=ps_s[:, :B2 * E], lhsT=ustrict[:], rhs=mask01[:],
                             start=True, stop=True)
            ps_c = ps.tile([P, 512], FP32, tag="ps")
            nc.tensor.matmul(out=ps_c[:, :B2 * E], lhsT=ones_t[:], rhs=mask01[:],
                             start=True, stop=True)
            ea = rt.tile([P, B2 * E], FP32)
            eb2 = rt.tile([P, B2 * E], FP32)
            nc.vector.memset(ea[:, :E], 0.0)
            nc.vector.tensor_copy(out=ea[:, E:], in_=ps_c[:, :(B2 - 1) * E])
            cur, nxt = ea, eb2
            s = 1
            while s < B2:
                w = s * E
                nc.vector.tensor_copy(out=nxt[:, :w], in_=cur[:, :w])
                nc.vector.tensor_tensor(out=nxt[:, w:B2 * E], in0=cur[:, w:B2 * E],
                                        in1=cur[:, :B2 * E - w], op=ALU.add)
                cur, nxt = nxt, cur
                s *= 2
            rnk = rt.tile([P, B2 * E], FP32)
            nc.vector.tensor_tensor(out=rnk[:], in0=ps_s[:, :B2 * E], in1=cur[:],
                                    op=ALU.add)

            # dense per-token weight for this core's expert:
            #   wden[n] = sum_k vw[k, n] * mask01[k, n, e0] * (rank < CAP)
            klt = rt.tile([P, B2 * E], FP32)
            nc.vector.tensor_scalar(out=klt[:], in0=rnk[:], scalar1=float(CAP),
                                    scalar2=None, op0=ALU.is_lt)
            kept = rt.tile([P, B2 * E], FP32)
            nc.vector.tensor_tensor(out=kept[:], in0=klt[:],
                                    in1=mask01[:].rearrange("p b e -> p (b e)"),
                                    op=ALU.mult)
            ksel = rt.tile([P, B2 * E], FP32)
            nc.vector.tensor_tensor(out=ksel[:], in0=kept[:], in1=es_sb[:], op=ALU.mult)
            ks2 = rt.tile([P, B2], FP32)
            nc.vector.tensor_reduce(out=ks2[:], in_=ksel[:].rearrange("p (b e) -> p b e", e=E),
                                    axis=X, op=ALU.add)
            wdb = rt.tile([P, B2], FP32)
            nc.vector.tensor_tensor(out=wdb[:], in0=ks2[:], in1=vw[:], op=ALU.mult)
            nc.vector.tensor_tensor(out=wden[:], in0=wdb[:, :NCH], in1=wdb[:, NCH:],
                                    op=ALU.add)
        if debug_taps:
            nc.sync.dma_start(out=dbg_wden[:], in_=wden[:])

        # ---------------- expert pipeline pools (reuse router space) -------
        mn = root.enter_context(tc.tile_pool(name="mn", bufs=1))
        lnp = root.enter_context(tc.tile_pool(name="ln", bufs=1))
        hT = mn.tile([P, FCH, TB], BF16)
        lnw_sb = lnp.tile([P, D], BF16)
        nc.sync.dma_start(out=lnw_sb[:], in_=lnw[:])
        lnb_sb = lnp.tile([P, D], BF16)
        nc.sync.dma_start(out=lnb_sb[:], in_=lnb[:])
        epsb = lnp.tile([P, 1], FP32)
        nc.vector.memset(epsb[:], float(ln_eps))

        def emit_ln(tb):
            """LayerNorm of this core's SH-row shard of block tb."""
            xr = lnp.tile([P, D], BF16, tag="xr", name="xr")
            nc.sync.dma_start(out=xr[:SH, :], in_=rs_o[tb][:])
            sm = lnp.tile([P, 1], FP32, tag="sm", name="sm")
            nc.vector.tensor_reduce(out=sm[:SH], in_=xr[:SH, :], axis=X, op=ALU.add)
            mu = lnp.tile([P, 1], FP32, tag="mu", name="mu")
            nc.vector.tensor_scalar(out=mu[:SH], in0=sm[:SH], scalar1=1.0 / D,
                                    scalar2=None, op0=ALU.mult)
            xc = lnp.tile([P, D], FP32, tag="xc", name="xc")
            nc.vector.tensor_scalar(out=xc[:SH], in0=xr[:SH, :], scalar1=mu[:SH],
                                    scalar2=None, op0=ALU.subtract)
            vs = lnp.tile([P, 1], FP32, tag="vs", name="vs")
            yo = lnp.tile([P, D], FP32, tag="yo", name="yo")
            nc.scalar.activation(out=yo[:SH, :], in_=xc[:SH], func=AF.Square,
                                 accum_out=vs[:SH])
            vr = lnp.tile([P, 1], FP32, tag="vr", name="vr")
            nc.vector.tensor_scalar(out=vr[:SH], in0=vs[:SH], scalar1=1.0 / D,
                                    scalar2=None, op0=ALU.mult)
            sd = lnp.tile([P, 1], FP32, tag="sd", name="sd")
            nc.scalar.activation(out=sd[:SH], in_=vr[:SH], func=AF.Sqrt,
                                 bias=epsb[:SH])
            rsd = lnp.tile([P, 1], FP32, tag="rsd", name="rsd")
            nc.vector.reciprocal(out=rsd[:SH], in_=sd[:SH])
            nc.vector.tensor_scalar(out=yo[:SH], in0=xc[:SH], scalar1=rsd[:SH],
                                    scalar2=None, op0=ALU.mult)
            nc.vector.tensor_tensor(out=yo[:SH], in0=yo[:SH], in1=lnw_sb[:SH, :],
                                    op=ALU.mult)
            nc.vector.tensor_tensor(out=yo[:SH], in0=yo[:SH], in1=lnb_sb[:SH, :],
                                    op=ALU.add)
            nc.sync.dma_start(out=out_ext[tb * SH:(tb + 1) * SH, :], in_=yo[:SH, :])

        # ---------------- dense expert compute (bf16) ----------------
        for tb in range(NTB):
            if tb + 1 < NTB:
                xb_t[tb + 1] = xbp.tile([P, KD, TB], BF16, tag="xb", bufs=2,
                                        name=f"xb{tb + 1}")
                nc.sync.dma_start(out=xb_t[tb + 1][:], in_=xbh[:, tb + 1])
            xb = xb_t[tb]
            # mm1: hT = act(c_fc^T-contraction with xT), f-major
            for f in range(FCH):
                hps = ps.tile([P, 512], FP32, tag="ps", name="hps")
                for kd in range(KD):
                    nc.tensor.matmul(out=hps[:, :HHW], lhsT=cfc_sb[:, kd, f],
                                     rhs=xb[:, kd], start=(kd == 0),
                                     stop=(kd == KD - 1))
                nc.scalar.activation(out=hT[:, f, :], in_=hps[:, :HHW], func=act_fn)
            # mm2: eo accumulated over f in 8 PSUM banks (4 m-chunks x 2 halves)
            eops = [ps.tile([P, 512], FP32, tag="ps", name=f"eops{i}")
                    for i in range(MCH * NDH)]
            for f in range(FCH):
                for m in range(MCH):
                    for dh in range(NDH):
                        nc.tensor.matmul(out=eops[m * NDH + dh][:, :DHW],
                                         lhsT=hT[:, f, m * P:(m + 1) * P],
                                         rhs=cpj_sb[:, f, dh * DHW:(dh + 1) * DHW],
                                         start=(f == 0), stop=(f == FCH - 1))
            for m in range(MCH):
                g = tb * MCH + m
                eo = mn.tile([P, D], BF16, tag="eo", bufs=2, name="eo")
                for dh in range(NDH):
                    nc.vector.tensor_tensor(
                        out=eo[:, dh * DHW:(dh + 1) * DHW],
                        in0=eops[m * NDH + dh][:, :DHW],
                        in1=wden[:, g:g + 1].to_broadcast([P, DHW]),
                        op=ALU.mult)
                nc.sync.dma_start(out=partial_b[tb][m * P:(m + 1) * P, :], in_=eo[:])
            if debug_taps:
                for m in range(MCH):
                    nc.sync.dma_start(
                        out=dbg_partial[(tb * MCH + m) * P:(tb * MCH + m + 1) * P, :],
                        in_=partial_b[tb][m * P:(m + 1) * P, :])
            nc.gpsimd.collective_compute(
                "ReduceScatter", mybir.AluOpType.add,
                replica_groups=[list(range(n_cores))],
                ins=[partial_b[tb].opt()], outs=[rs_o[tb].opt()])
            # LN for the PREVIOUS block (keeps the scalar/vector queues from
            # blocking on this block's RS while the next block computes)
            if tb > 0:
                emit_ln(tb - 1)
        emit_ln(NTB - 1)

    nc.compile()
    return nc


def prep_in_maps(x, w_g, c_fc, c_proj, ln_w, ln_b, cfg):
    """Host-side input prep: replication, layout tiling, bf16 cast."""
    from concourse import mybir

    N, D, E, CAP, TB = cfg["N"], cfg["D"], cfg["E"], cfg["CAP"], cfg["TB"]
    n_cores = cfg["n_cores"]
    F = 4 * D
    KD, FCH = D // P, F // P
    NCH = N // P
    B2 = 2 * NCH
    NTB = N // TB
    bf16 = mybir.dt.np(mybir.dt.bfloat16)

    xf = np.ascontiguousarray(np.asarray(x, np.float32).reshape(N, D))
    xT = np.ascontiguousarray(xf.T)
    xbh = np.ascontiguousarray(
        xT.reshape(KD, P, NTB, TB).transpose(1, 2, 0, 3)).astype(bf16)
    wg = np.ascontiguousarray(np.asarray(w_g, np.float32))
    cfc_all = np.asarray(c_fc, np.float32)
    cpj_all = np.asarray(c_proj, np.float32)
    lnw = np.ascontiguousarray(
        np.broadcast_to(np.asarray(ln_w, np.float32), (P, D))).astype(bf16)
    lnb = np.ascontiguousarray(
        np.broadcast_to(np.asarray(ln_b, np.float32), (P, D))).astype(bf16)

    in_maps = []
    for e in range(n_cores):
        cfc_t = np.ascontiguousarray(
            cfc_all[e].reshape(KD, P, FCH, P).transpose(1, 0, 2, 3)).astype(bf16)
        cpj_t = np.ascontiguousarray(
            cpj_all[e].reshape(FCH, P, D).transpose(1, 0, 2)).astype(bf16)
        ev = np.zeros((E,), np.float32)
        ev[e] = 1.0
        esel = np.ascontiguousarray(
            np.broadcast_to(np.tile(ev, B2), (P, B2 * E)))
        in_maps.append(dict(xT=xT, wg=wg, xbh=xbh, cfc=cfc_t, cpj=cpj_t,
                            esel=esel, lnw=lnw, lnb=lnb))
    return in_maps


_CACHE = {}


def _compiled_full():
    key = "full"
    if key not in _CACHE:
        _CACHE[key] = build_moe_kernel(**FULL_CFG)
    return _CACHE[key]


def run_on_hw(inputs, trace=False):
    """Runs the full-size kernel on the 8 NeuronCores. Returns (out, results)."""
    from concourse.bass_utils import run_bass_kernel_spmd

    cfg = FULL_CFG
    nc = _compiled_full()
    in_maps = prep_in_maps(inputs["x"], inputs["w_g"], inputs["c_fc"],
                           inputs["c_proj"], inputs["ln_w"], inputs["ln_b"], cfg)
    res = run_bass_kernel_spmd(nc, in_maps, list(range(cfg["n_cores"])),
                               trace=trace)
    N, D, TB = cfg["N"], cfg["D"], cfg["TB"]
    NTB = N // TB
    SH = TB // cfg["n_cores"]
    shards = np.stack([res.results[i]["out"] for i in range(cfg["n_cores"])])
    out = shards.reshape(cfg["n_cores"], NTB, SH, D).transpose(1, 0, 2, 3)
    out = np.ascontiguousarray(out.reshape(N, D)).astype(np.float32)
    B, T = 4, 1024
    return out.reshape(B, T, D), res


def kernel(x, w_g, c_fc, c_proj, ln_w, ln_b):
    out, _ = run_on_hw(dict(x=x, w_g=w_g, c_fc=c_fc, c_proj=c_proj,
                            ln_w=ln_w, ln_b=ln_b))
    return out


# revision 5
# speedup vs baseline: 3.9758x; 1.7384x over previous
"""Trainium2 Bass kernel for a DeepSeek-style MoE block (expert-parallel over 8 cores).

Strategy (dense expert-parallel, bf16 expert compute):
  - Each core owns one expert (8 experts / 8 cores). x (transposed) + router
    weights are replicated; c_fc/c_proj are sharded along the expert axis.
  - Every core computes the full router on-device in fp32: logits -> top-2 ->
    softmax -> capacity ranking (exclusive cumsum over the flattened
    (slot, token) order via a strictly-triangular matmul + log-step block
    scan). The result is a dense per-token weight vector for this core's
    expert (0 for tokens not routed here or dropped by capacity).
  - Expert compute runs DENSELY over all 4096 tokens in bf16 (2x the routed
    FLOPs, but no gathers/scatters; bf16 runs the PE at 1 cycle/row vs 4 for
    fp32). Both weight matrices live SBUF-resident (8 MB + 8 MB bf16), loaded
    once. Tokens stream through in 8 blocks of 512.
  - The per-token weight is applied to the expert output, cast to bf16, and
    written densely to a per-block [512, D] partial buffer. A per-block
    ReduceScatter (bf16) combines partials across the 8 cores while later
    blocks still compute; each core LayerNorms its 64-row shard of every
    block. The host reassembles the full output.

Matmul orientation keeps activations feature-major so both weights are used
in their native layout:
  hT[f, t] = sum_d c_fc[d, f] * xT[d, t]       (lhsT = c_fc slab, rhs = xT)
  eo[t, d] = sum_f hT[f, t] * c_proj[f, d]     (lhsT = hT slice,  rhs = c_proj)
"""

import os
import sys
from contextlib import ExitStack

import numpy as np

for _p in ("/opt/trn_rl_repo", "/root/.axon_site/_ro/trn_rl_repo"):
    if os.path.isdir(_p) and _p not in sys.path:
        sys.path.insert(0, _p)

P = 128

FULL_CFG = dict(N=4096, D=1024, E=8, CAP=2048, TB=512, n_cores=8,
                act="Gelu", ln_eps=1e-5)


def build_moe_kernel(N, D, E, CAP, TB, n_cores, act="Gelu", ln_eps=1e-5,
                     debug_taps=False):
    """Builds and compiles the SPMD Bass kernel. Returns the Bacc object."""
    from concourse import bacc, bass, mybir
    import concourse.tile as tile
    from concourse.masks import make_identity, make_upper_triangular

    FP32 = mybir.dt.float32
    BF16 = mybir.dt.bfloat16
    AF = mybir.ActivationFunctionType
    ALU = mybir.AluOpType
    X = mybir.AxisListType.X

    F = 4 * D
    NCH = N // P           # token chunks (128 tokens each)
    KD = D // P            # contraction chunks for mm1
    FCH = F // P           # f chunks
    B2 = 2 * NCH           # (slot k, token-chunk) columns in rank order
    NTB = N // TB          # token blocks for the expert pipeline
    MCH = TB // P          # token chunks per block
    DHW = min(512, D)      # mm2 output width per matmul
    NDH = D // DHW
    HHW = min(512, TB)     # mm1 output width per matmul
    NHH = TB // HHW
    SH = TB // n_cores     # RS shard rows per core per block
    NB512 = N // 512       # router column blocks
    act_fn = getattr(AF, act)
    assert N % 512 == 0 and B2 * E <= 512 and NHH == 1

    nc = bacc.Bacc("TRN2", target_bir_lowering=False, debug=False,
                   num_devices=n_cores)

    xT = nc.dram_tensor("xT", [D, N], FP32, kind="ExternalInput").ap()
    wg = nc.dram_tensor("wg", [D, E], FP32, kind="ExternalInput").ap()
    xbh = nc.dram_tensor("xbh", [P, NTB, KD, TB], BF16, kind="ExternalInput").ap()
    cfc = nc.dram_tensor("cfc", [P, KD, FCH, P], BF16, kind="ExternalInput").ap()
    cpj = nc.dram_tensor("cpj", [P, FCH, D], BF16, kind="ExternalInput").ap()
    esel = nc.dram_tensor("esel", [P, B2 * E], FP32, kind="ExternalInput").ap()
    lnw = nc.dram_tensor("lnw", [P, D], BF16, kind="ExternalInput").ap()
    lnb = nc.dram_tensor("lnb", [P, D], BF16, kind="ExternalInput").ap()
    out_ext = nc.dram_tensor("out", [NTB * SH, D], FP32, kind="ExternalOutput").ap()
    if debug_taps:
        dbg_wden = nc.dram_tensor("dbg_wden", [P, NCH], FP32,
                                  kind="ExternalOutput").ap()
        dbg_partial = nc.dram_tensor("dbg_partial", [N, D], FP32,
                                     kind="ExternalOutput").ap()

    with tile.TileContext(nc) as tc:
      with ExitStack() as root:
        dram = root.enter_context(tc.tile_pool(name="dram", bufs=1, space="DRAM"))
        ps = root.enter_context(tc.tile_pool(name="ps", bufs=8, space="PSUM"))
        const = root.enter_context(tc.tile_pool(name="const", bufs=1))
        wts = root.enter_context(tc.tile_pool(name="wts", bufs=1))
        xbp = root.enter_context(tc.tile_pool(name="xbp", bufs=1))

        partial_b = [dram.tile([TB, D], BF16, name=f"partialb{tb}",
                               tag=f"pb{tb}") for tb in range(NTB)]
        rs_o = [dram.tile([SH, D], BF16, name=f"rso{tb}", tag=f"rs{tb}")
                for tb in range(NTB)]

        ident = const.tile([P, P], FP32)
        make_identity(nc, ident[:])
        ustrict = const.tile([P, P], FP32)   # U[k, m] = 1 iff m > k
        make_upper_triangular(nc, ustrict[:], val=1.0, diag=False)
        ones_t = const.tile([P, P], FP32)
        nc.vector.memset(ones_t[:], 1.0)
        wden = const.tile([P, NCH], FP32)    # per-token weight, this expert

        # resident weight tiles (loads issued after the router's first chunk
        # of DMAs so the router matmuls start immediately)
        cfc_sb = wts.tile([P, KD, FCH, P], BF16, tag="cfc")
        cpj_sb = wts.tile([P, FCH, D], BF16, tag="cpj")
        xb_t = [None] * NTB

        def load_weights_and_xb0():
            for kd in range(KD):
                nc.sync.dma_start(out=cfc_sb[:, kd], in_=cfc[:, kd])
            xb_t[0] = xbp.tile([P, KD, TB], BF16, tag="xb", bufs=2, name="xb0")
            nc.sync.dma_start(out=xb_t[0][:], in_=xbh[:, 0])
            for fg in range(FCH // 2):
                nc.sync.dma_start(out=cpj_sb[:, fg * 2:(fg + 1) * 2],
                                  in_=cpj[:, fg * 2:(fg + 1) * 2])

        load_weights_and_xb0()

        # ---------------- router (fp32, scoped pool) ----------------
        # router DMAs ride the Activation-engine HWDGE queue so they are not
        # stuck behind the weight prefetch on the Sync queue's rings
        with tc.tile_pool(name="rt", bufs=1) as rt:
            wg_sb = rt.tile([P, KD, E], FP32)
            nc.scalar.dma_start(out=wg_sb[:], in_=wg.rearrange("(k p) e -> p k e", p=P))
            es_sb = rt.tile([P, B2 * E], FP32)
            nc.scalar.dma_start(out=es_sb[:], in_=esel[:])

            # logits[n, e] computed as (w_g^T @ x^T)^T in 512-token blocks
            logits = rt.tile([P, NCH, E], FP32)
            for nb in range(NB512):
                ps_lt = ps.tile([P, 512], FP32, tag="ps")
                for k in range(KD):
                    xt_sb = rt.tile([P, 512], FP32, tag="xt", bufs=3)
                    nc.scalar.dma_start(out=xt_sb[:], in_=xT[k * P:(k + 1) * P,
                                                            nb * 512:(nb + 1) * 512])
                    nc.tensor.matmul(out=ps_lt[:E, :], lhsT=wg_sb[:, k, :],
                                     rhs=xt_sb[:], start=(k == 0), stop=(k == KD - 1))
                lt_sb = rt.tile([E, 512], FP32, tag="lt", bufs=2)
                nc.vector.tensor_copy(out=lt_sb[:], in_=ps_lt[:E, :])
                for i in range(4):  # 512 tokens -> 4 chunks of 128
                    ps_t = ps.tile([P, 512], FP32, tag="ps")
                    nc.tensor.transpose(out=ps_t[:, :E], in_=lt_sb[:, i * P:(i + 1) * P],
                                        identity=ident[:E, :E])
                    nc.vector.tensor_copy(out=logits[:, nb * 4 + i, :], in_=ps_t[:, :E])

            # top-2 over experts
            v0 = rt.tile([P, NCH], FP32)
            nc.vector.tensor_reduce(out=v0[:], in_=logits[:], axis=X, op=ALU.max)
            mask01 = rt.tile([P, B2, E], FP32)
            nc.vector.tensor_tensor(out=mask01[:, :NCH, :], in0=logits[:],
                                    in1=v0[:].unsqueeze(2).to_broadcast([P, NCH, E]),
                                    op=ALU.is_equal)
            mbig = rt.tile([P, NCH, E], FP32)
            nc.vector.tensor_scalar(out=mbig[:], in0=mask01[:, :NCH, :],
                                    scalar1=1e30, scalar2=None, op0=ALU.mult)
            lm = rt.tile([P, NCH, E], FP32)
            nc.vector.tensor_tensor(out=lm[:], in0=logits[:], in1=mbig[:], op=ALU.subtract)
            v1 = rt.tile([P, NCH], FP32)
            nc.vector.tensor_reduce(out=v1[:], in_=lm[:], axis=X, op=ALU.max)
            nc.vector.tensor_tensor(out=mask01[:, NCH:, :], in0=lm[:],
                                    in1=v1[:].unsqueeze(2).to_broadcast([P, NCH, E]),
                                    op=ALU.is_equal)

            # softmax over the two selected logits
            dv = rt.tile([P, NCH], FP32)
            nc.vector.tensor_tensor(out=dv[:], in0=v1[:], in1=v0[:], op=ALU.subtract)
            p1 = rt.tile([P, NCH], FP32)
            nc.scalar.activation(out=p1[:], in_=dv[:], func=AF.Exp)
            z = rt.tile([P, NCH], FP32)
            nc.vector.tensor_scalar(out=z[:], in0=p1[:], scalar1=1.0, scalar2=None,
                                    op0=ALU.add)
            vw = rt.tile([P, B2], FP32)
            w0v = rt.tile([P, NCH], FP32)
            nc.vector.reciprocal(out=w0v[:], in_=z[:])
            nc.vector.tensor_copy(out=vw[:, :NCH], in_=w0v[:])
            nc.vector.tensor_tensor(out=vw[:, NCH:], in0=p1[:], in1=w0v[:], op=ALU.mult)

            # exclusive cumsum over flattened (k, n) per expert:
            # intra-chunk via strictly-upper-triangular matmul, chunk offsets
            # via a log-step scan over per-chunk column sums
            ps_s = ps.tile([P, 512], FP32, tag="ps")
            nc.tensor.matmul(out=ps_s[:, :B2 * E], lhsT=ustrict[:], rhs=mask01[:],
                             start=True, stop=True)
            ps_c = ps.tile([P, 512], FP32, tag="ps")
            nc.tensor.matmul(out=ps_c[:, :B2 * E], lhsT=ones_t[:], rhs=mask01[:],
                             start=True, stop=True)
            ea = rt.tile([P, B2 * E], FP32)
            eb2 = rt.tile([P, B2 * E], FP32)
            nc.vector.memset(ea[:, :E], 0.0)
            nc.vector.tensor_copy(out=ea[:, E:], in_=ps_c[:, :(B2 - 1) * E])
            cur, nxt = ea, eb2
            s = 1
            while s < B2:
                w = s * E
                nc.vector.tensor_copy(out=nxt[:, :w], in_=cur[:, :w])
                nc.vector.tensor_tensor(out=nxt[:, w:B2 * E], in0=cur[:, w:B2 * E],
                                        in1=cur[:, :B2 * E - w], op=ALU.add)
                cur, nxt = nxt, cur
                s *= 2
            rnk = rt.tile([P, B2 * E], FP32)
            nc.vector.tensor_tensor(out=rnk[:], in0=ps_s[:, :B2 * E], in1=cur[:],
                                    op=ALU.add)

            # dense per-token weight for this core's expert:
            #   wden[n] = sum_k vw[k, n] * mask01[k, n, e0] * (rank < CAP)
            klt = rt.tile([P, B2 * E], FP32)
            nc.vector.tensor_scalar(out=klt[:], in0=rnk[:], scalar1=float(CAP),
                                    scalar2=None, op0=ALU.is_lt)
            kept = rt.tile([P, B2 * E], FP32)
            nc.vector.tensor_tensor(out=kept[:], in0=klt[:],
                                    in1=mask01[:].rearrange("p b e -> p (b e)"),
                                    op=ALU.mult)
            ksel = rt.tile([P, B2 * E], FP32)
            nc.vector.tensor_tensor(out=ksel[:], in0=kept[:], in1=es_sb[:], op=ALU.mult)
            ks2 = rt.tile([P, B2], FP32)
            nc.vector.tensor_reduce(out=ks2[:], in_=ksel[:].rearrange("p (b e) -> p b e", e=E),
                                    axis=X, op=ALU.add)
            wdb = rt.tile([P, B2], FP32)
            nc.vector.tensor_tensor(out=wdb[:], in0=ks2[:], in1=vw[:], op=ALU.mult)
            nc.vector.tensor_tensor(out=wden[:], in0=wdb[:, :NCH], in1=wdb[:, NCH:],
                                    op=ALU.add)
        if debug_taps:
            nc.sync.dma_start(out=dbg_wden[:], in_=wden[:])

        # ---------------- expert pipeline pools (reuse router space) -------
        mn = root.enter_context(tc.tile_pool(name="mn", bufs=1))
        lnp = root.enter_context(tc.tile_pool(name="ln", bufs=1))
        hT = mn.tile([P, FCH, TB], BF16)
        lnw_sb = lnp.tile([P, D], BF16)
        nc.sync.dma_start(out=lnw_sb[:], in_=lnw[:])
        lnb_sb = lnp.tile([P, D], BF16)
        nc.sync.dma_start(out=lnb_sb[:], in_=lnb[:])
        epsb = lnp.tile([P, 1], FP32)
        nc.vector.memset(epsb[:], float(ln_eps))

        def emit_ln(tb):
            """LayerNorm of this core's SH-row shard of block tb."""
            xr = lnp.tile([P, D], BF16, tag="xr", name="xr")
            nc.sync.dma_start(out=xr[:SH, :], in_=rs_o[tb][:])
            sm = lnp.tile([P, 1], FP32, tag="sm", name="sm")
            nc.vector.tensor_reduce(out=sm[:SH], in_=xr[:SH, :], axis=X, op=ALU.add)
            mu = lnp.tile([P, 1], FP32, tag="mu", name="mu")
            nc.vector.tensor_scalar(out=mu[:SH], in0=sm[:SH], scalar1=1.0 / D,
                                    scalar2=None, op0=ALU.mult)
            xc = lnp.tile([P, D], FP32, tag="xc", name="xc")
            nc.vector.tensor_scalar(out=xc[:SH], in0=xr[:SH, :], scalar1=mu[:SH],
                                    scalar2=None, op0=ALU.subtract)
            vs = lnp.tile([P, 1], FP32, tag="vs", name="vs")
            yo = lnp.tile([P, D], FP32, tag="yo", name="yo")
            nc.scalar.activation(out=yo[:SH, :], in_=xc[:SH], func=AF.Square,
                                 accum_out=vs[:SH])
            vr = lnp.tile([P, 1], FP32, tag="vr", name="vr")
            nc.vector.tensor_scalar(out=vr[:SH], in0=vs[:SH], scalar1=1.0 / D,
                                    scalar2=None, op0=ALU.mult)
            sd = lnp.tile([P, 1], FP32, tag="sd", name="sd")
            nc.scalar.activation(out=sd[:SH], in_=vr[:SH], func=AF.Sqrt,
                                 bias=epsb[:SH])
            rsd = lnp.tile([P, 1], FP32, tag="rsd", name="rsd")
            nc.vector.reciprocal(out=rsd[:SH], in_=sd[:SH])
            nc.vector.tensor_scalar(out=yo[:SH], in0=xc[:SH], scalar1=rsd[:SH],
                                    scalar2=None, op0=ALU.mult)
            nc.vector.tensor_tensor(out=yo[:SH], in0=yo[:SH], in1=lnw_sb[:SH, :],
                                    op=ALU.mult)
            nc.vector.tensor_tensor(out=yo[:SH], in0=yo[:SH], in1=lnb_sb[:SH, :],
                                    op=ALU.add)
            nc.sync.dma_start(out=out_ext[tb * SH:(tb + 1) * SH, :], in_=yo[:SH, :])

        # ---------------- dense expert compute (bf16) ----------------
        for tb in range(NTB):
            if tb + 1 < NTB:
                xb_t[tb + 1] = xbp.tile([P, KD, TB], BF16, tag="xb", bufs=2,
                                        name=f"xb{tb + 1}")
                nc.sync.dma_start(out=xb_t[tb + 1][:], in_=xbh[:, tb + 1])
            xb = xb_t[tb]
            # mm1: hT = act(c_fc^T-contraction with xT), f-major
            for f in range(FCH):
                hps = ps.tile([P, 512], FP32, tag="ps", name="hps")
                for kd in range(KD):
                    nc.tensor.matmul(out=hps[:, :HHW], lhsT=cfc_sb[:, kd, f],
                                     rhs=xb[:, kd], start=(kd == 0),
                                     stop=(kd == KD - 1))
                nc.scalar.activation(out=hT[:, f, :], in_=hps[:, :HHW], func=act_fn)
            # mm2: eo accumulated over f in 8 PSUM banks (4 m-chunks x 2 halves)
            eops = [ps.tile([P, 512], FP32, tag="ps", name=f"eops{i}")
                    for i in range(MCH * NDH)]
            for f in range(FCH):
                for m in range(MCH):
                    for dh in range(NDH):
                        nc.tensor.matmul(out=eops[m * NDH + dh][:, :DHW],
                                         lhsT=hT[:, f, m * P:(m + 1) * P],
                                         rhs=cpj_sb[:, f, dh * DHW:(dh + 1) * DHW],
                                         start=(f == 0), stop=(f == FCH - 1))
            for m in range(MCH):
                g = tb * MCH + m
                eo = mn.tile([P, D], BF16, tag="eo", bufs=2, name="eo")
                for dh in range(NDH):
                    nc.vector.tensor_tensor(
                        out=eo[:, dh * DHW:(dh + 1) * DHW],
                        in0=eops[m * NDH + dh][:, :DHW],
                        in1=wden[:, g:g + 1].to_broadcast([P, DHW]),
                        op=ALU.mult)
                nc.sync.dma_start(out=partial_b[tb][m * P:(m + 1) * P, :], in_=eo[:])
            if debug_taps:
                for m in range(MCH):
                    nc.sync.dma_start(
                        out=dbg_partial[(tb * MCH + m) * P:(tb * MCH + m + 1) * P, :],
                        in_=partial_b[tb][m * P:(m + 1) * P, :])
            nc.gpsimd.collective_compute(
                "ReduceScatter", mybir.AluOpType.add,
                replica_groups=[list(range(n_cores))],
                ins=[partial_b[tb].opt()], outs=[rs_o[tb].opt()])
            # LN for the PREVIOUS block (keeps the scalar/vector queues from
            # blocking on this block's RS while the next block computes)
            if tb > 0:
                emit_ln(tb - 1)
        emit_ln(NTB - 1)

    nc.compile()
    return nc


def prep_in_maps(x, w_g, c_fc, c_proj, ln_w, ln_b, cfg):
    """Host-side input prep: replication, layout tiling, bf16 cast."""
    from concourse import mybir

    N, D, E, CAP, TB = cfg["N"], cfg["D"], cfg["E"], cfg["CAP"], cfg["TB"]
    n_cores = cfg["n_cores"]
    F = 4 * D
    KD, FCH = D // P, F // P
    NCH = N // P
    B2 = 2 * NCH
    NTB = N // TB
    bf16 = mybir.dt.np(mybir.dt.bfloat16)

    xf = np.ascontiguousarray(np.asarray(x, np.float32).reshape(N, D))
    xT = np.ascontiguousarray(xf.T)
    xbh = np.ascontiguousarray(
        xT.reshape(KD, P, NTB, TB).transpose(1, 2, 0, 3)).astype(bf16)
    wg = np.ascontiguousarray(np.asarray(w_g, np.float32))
    cfc_all = np.asarray(c_fc, np.float32)
    cpj_all = np.asarray(c_proj, np.float32)
    lnw = np.ascontiguousarray(
        np.broadcast_to(np.asarray(ln_w, np.float32), (P, D))).astype(bf16)
    lnb = np.ascontiguousarray(
        np.broadcast_to(np.asarray(ln_b, np.float32), (P, D))).astype(bf16)

    in_maps = []
    for e in range(n_cores):
        cfc_t = np.ascontiguousarray(
            cfc_all[e].reshape(KD, P, FCH, P).transpose(1, 0, 2, 3)).astype(bf16)
        cpj_t = np.ascontiguousarray(
            cpj_all[e].reshape(FCH, P, D).transpose(1, 0, 2)).astype(bf16)
        ev = np.zeros((E,), np.float32)
        ev[e] = 1.0
        esel = np.ascontiguousarray(
            np.broadcast_to(np.tile(ev, B2), (P, B2 * E)))
        in_maps.append(dict(xT=xT, wg=wg, xbh=xbh, cfc=cfc_t, cpj=cpj_t,
                            esel=esel, lnw=lnw, lnb=lnb))
    return in_maps


_CACHE = {}


def _compiled_full():
    key = "full"
    if key not in _CACHE:
        _CACHE[key] = build_moe_kernel(**FULL_CFG)
    return _CACHE[key]


def run_on_hw(inputs, trace=False):
    """Runs the full-size kernel on the 8 NeuronCores. Returns (out, results)."""
    from concourse.bass_utils import run_bass_kernel_spmd

    cfg = FULL_CFG
    nc = _compiled_full()
    in_maps = prep_in_maps(inputs["x"], inputs["w_g"], inputs["c_fc"],
                           inputs["c_proj"], inputs["ln_w"], inputs["ln_b"], cfg)
    res = run_bass_kernel_spmd(nc, in_maps, list(range(cfg["n_cores"])),
                               trace=trace)
    N, D, TB = cfg["N"], cfg["D"], cfg["TB"]
    NTB = N // TB
    SH = TB // cfg["n_cores"]
    shards = np.stack([res.results[i]["out"] for i in range(cfg["n_cores"])])
    out = shards.reshape(cfg["n_cores"], NTB, SH, D).transpose(1, 0, 2, 3)
    out = np.ascontiguousarray(out.reshape(N, D)).astype(np.float32)
    B, T = 4, 1024
    return out.reshape(B, T, D), res


def kernel(x, w_g, c_fc, c_proj, ln_w, ln_b):
    out, _ = run_on_hw(dict(x=x, w_g=w_g, c_fc=c_fc, c_proj=c_proj,
                            ln_w=ln_w, ln_b=ln_b))
    return out
